# revision 17
# baseline (speedup 1.0000x reference)
"""Causal single-head attention (B=4, T=4096, C=1024, H=64) on 8 TRN2 NeuronCores.

Sharding: 2 cores per batch element; within a batch, the 8 query blocks of 512
rows are split by parity (core s owns blocks {s, s+2, s+4, s+6}).

v2.2:
  - x / weights cast f32->f16 in-flight by SWDGE DMA (no engine cast cost).
  - Attention is software-pipelined per kv PAIR: both S^T matmuls of a pair
    write one 2-bank PSUM tile, ONE exp covers the pair (fewer ACT
    instructions and semaphore round-trips on the critical recycle loop),
    then one fp8e4 DoubleRow P@V (far pairs) or two f16 P@Vs (near pairs,
    one paired causal-mask multiply split DVE/GpSimd).
  - V transposes ride the DMA X-bar (SBUF->SBUF transpose DMA) instead of
    PE transposes + DVE copies; the freed PSUM bank deepens the S pipeline.
  - PV emission lags S by 2 pairs round-robin across blocks so the PE always
    has ready work and the HAM clock stays warm; at most 2 blocks hold open
    PSUM accumulators (2 po banks).
"""

import numpy as np

import concourse.bacc as bacc
import concourse.bass as bass
import concourse.mybir as mybir
import concourse.tile as tile
from concourse.bass_utils import run_bass_kernel_spmd
from concourse.masks import make_identity

B, T, C, H = 4, 4096, 1024, 64
NCORES = 8
TB = 512                 # q/t block width
NTB = T // TB            # 8 t-blocks for projections
NQB = 4                  # local q blocks per core
NKVT = T // 128          # 32 kv tiles of 128
F32 = mybir.dt.float32
F16 = mybir.dt.float16
F8 = mybir.dt.float8e4

_nc = None


def _build():
    nc = bacc.Bacc("TRN2", target_bir_lowering=False, debug=False, num_devices=NCORES)
    xt = nc.dram_tensor("xt", [C, T], F32, kind="ExternalInput").ap()
    wq = nc.dram_tensor("wq", [128, 8 * H], F32, kind="ExternalInput").ap()
    wkv = nc.dram_tensor("wkv", [128, 8 * 2 * H], F32, kind="ExternalInput").ap()
    masks = nc.dram_tensor("masks", [128, 8 * TB], F16, kind="ExternalInput").ap()
    out = nc.dram_tensor("out", [NQB * TB, H], F32, kind="ExternalOutput").ap()

    with tile.TileContext(nc) as tc:
        pid = nc.partition_id(engines=[mybir.EngineType.PE])
        s = pid % 2
        with tc.tile_pool(name="persist", bufs=1) as persist, \
             tc.tile_pool(name="x16p", bufs=16) as x16p, \
             tc.tile_pool(name="vtp", bufs=2) as vtp, \
             tc.tile_pool(name="otp", bufs=2) as otp, \
             tc.tile_pool(name="obp", bufs=3) as obp, \
             tc.tile_pool(name="rcp", bufs=2) as rcp, \
             tc.tile_pool(name="ptp", bufs=6) as ptp, \
             tc.tile_pool(name="pt8p", bufs=6) as pt8p, \
             tc.tile_pool(name="pjp", bufs=2, space="PSUM") as pj_pool, \
             tc.tile_pool(name="psp", bufs=4, space="PSUM") as ps_pool, \
             tc.tile_pool(name="pop", bufs=2, space="PSUM") as po_pool:
            ident16 = persist.tile([128, 128], F16)
            make_identity(nc, ident16)
            wq_sb = persist.tile([128, 8 * H], F16)
            wkv_sb = persist.tile([128, 8 * 2 * H], F16)
            nc.gpsimd.dma_start(out=wq_sb, in_=wq)
            nc.gpsimd.dma_start(out=wkv_sb, in_=wkv)
            masks_sb = persist.tile([128, 8 * TB], F16)
            nc.scalar.dma_start(out=masks_sb, in_=masks)

            QT = persist.tile([64, T], F16)           # Q^T on partitions 0:64
            KT = persist.tile([64, T], F16)           # K^T on partitions 0:64
            # 80-wide rows: X-bar transpose dest must be 32B-aligned (160B stride)
            V16 = persist.tile([128, NKVT, 80], F16)
            # DoubleRow weights need the Ko step to be a multiple of 16 bytes:
            # pad each kv tile's [V | ones] row to 80 fp8 bytes (65:80 zero).
            V8 = persist.tile([128, NKVT // 2, 2, 80], F8)
            nc.vector.memset(V8, 0.0)
            # col 64 of each kv tile = 1.0 (row-sum column)
            nc.scalar.activation(
                V16[:, :, H],
                ident16[:, 0:NKVT],
                mybir.ActivationFunctionType.Copy,
                scale=0.0,
                bias=1.0,
            )
            for b8 in range(2):
                nc.scalar.activation(
                    V8[:, :, b8, H],
                    ident16[:, 0:NKVT // 2],
                    mybir.ActivationFunctionType.Copy,
                    scale=0.0,
                    bias=1.0,
                )

            # pre-warm the PE clock while the first x DMAs are in flight
            for w in range(8):
                psum_warm = ps_pool.tile([128, TB], F32, name="psum_warm",
                                         tag="ps")
                nc.tensor.matmul(
                    psum_warm[:, 0:128], ident16, ident16,
                    start=True, stop=True,
                )

            qoffs = [s * TB + i * 2 * TB for i in range(NQB)]
            x16s = [None] * 8

            # ---- attention machinery: S-phase / PV-phase software pipeline ----
            st = {
                "next_s": [0] * NQB,    # next pair to emit S+exp for
                "next_pv": [0] * NQB,   # next pair to emit PV for
                "po": [None] * NQB,
                "pt": {},               # (i, kp) -> pt pair tile
            }

            def npairs(i):
                return (8 * i + 8) // 2

            def emit_s(i, kp):
                nkv = 8 * i + 8
                far = (2 * kp + 1) < (nkv - 8)
                if far:
                    pt8 = pt8p.tile([128, 2, TB], F8, name="pt8", tag="pt8")
                    st["pt"][(i, kp)] = pt8
                else:
                    st["pt"][(i, kp)] = []
                for h in range(2):
                    k = 2 * kp + h
                    psum_s = ps_pool.tile([128, TB], F32, name="psum_s",
                                          tag="ps")
                    nc.tensor.matmul(
                        psum_s,
                        KT[:, k * 128:(k + 1) * 128],
                        QT[:, bass.ds(qoffs[i], TB)],
                        start=True,
                        stop=True,
                    )
                    if far:
                        nc.scalar.activation(
                            pt8[:, h, :], psum_s,
                            mybir.ActivationFunctionType.Exp, scale=0.125
                        )
                    else:
                        pt = ptp.tile([128, TB], F16, name="pt", tag="pt")
                        nc.scalar.activation(
                            pt, psum_s, mybir.ActivationFunctionType.Exp,
                            scale=0.125
                        )
                        j = k - (nkv - 8)
                        if j >= 0:
                            eng = nc.gpsimd if j < 4 else nc.vector
                            eng.tensor_mul(
                                pt, pt, masks_sb[:, j * TB:(j + 1) * TB]
                            )
                        st["pt"][(i, kp)].append(pt)

            def emit_pv(i, kp):
                nkv = 8 * i + 8
                far = (2 * kp + 1) < (nkv - 8)
                if kp == 0:
                    st["po"][i] = po_pool.tile([80, TB], F32, name="psum_o",
                                               tag="po")
                psum_o = st["po"][i]
                pt = st["pt"].pop((i, kp))
                if far:
                    nc.tensor.matmul(
                        psum_o,
                        V8[:, kp, :, :],
                        pt,
                        start=(kp == 0),
                        stop=False,
                        perf_mode=mybir.MatmulPerfMode.DoubleRow,
                    )
                else:
                    for h in range(2):
                        k = 2 * kp + h
                        nc.tensor.matmul(
                            psum_o[0:H + 1, :],
                            V16[:, k, 0:H + 1],
                            pt[h],
                            start=(k == 0),
                            stop=(k == nkv - 1),
                        )
                if 2 * kp + 1 == nkv - 1:
                    # epilogue: normalize + store this q block
                    ot = otp.tile([H + 1, TB], F16)
                    nc.vector.tensor_copy(ot, psum_o[0:H + 1, :])
                    for j2 in range(4):
                        psum_t = po_pool.tile([128, H + 1], F32, name="psum_t",
                                              tag="po")
                        nc.tensor.matmul(
                            psum_t,
                            ot[:, j2 * 128:(j2 + 1) * 128],
                            ident16[0:H + 1, 0:H + 1],
                            start=True,
                            stop=True,
                        )
                        rec = rcp.tile([128, 1], F32)
                        nc.vector.reciprocal(rec, psum_t[:, H:H + 1])
                        ob = obp.tile([128, H], F32)
                        nc.vector.tensor_scalar_mul(ob, psum_t[:, 0:H], rec)
                        nc.sync.dma_start(
                            out=out[i * TB + j2 * 128:i * TB + (j2 + 1) * 128, :],
                            in_=ob,
                        )

            def avail_g(i, kp):
                # q block i needs QT global block 2i+s (<= 2i+1); kv pair kp
                # needs proj t-block (2kp+1)//4.
                base = max(2 * i + 1, (2 * kp + 1) // 4)
                if i == 2:
                    base = max(base, 6)
                return base

            def n_open():
                return sum(
                    1 for i in range(NQB)
                    if 0 < st["next_pv"][i] < npairs(i)
                )

            def emit_ready(g, budget):
                emitted = 1
                while budget != 0 and emitted:
                    emitted = 0
                    for i in range(NQB):
                        if budget == 0:
                            break
                        did = 0
                        opened = 0 < st["next_pv"][i] < npairs(i)
                        may_open = opened or n_open() < 2
                        if (st["next_s"][i] < npairs(i)
                                and avail_g(i, st["next_s"][i]) <= g
                                and (st["next_s"][i] - st["next_pv"][i] < 2
                                     or may_open)):
                            emit_s(i, st["next_s"][i])
                            st["next_s"][i] += 1
                            did = 1
                        while may_open and (
                            st["next_pv"][i] < st["next_s"][i] - 2
                            or (st["next_s"][i] == npairs(i)
                                and g >= NTB
                                and st["next_pv"][i] < st["next_s"][i])
                        ):
                            emit_pv(i, st["next_pv"][i])
                            st["next_pv"][i] += 1
                            opened = st["next_pv"][i] < npairs(i)
                            may_open = opened or n_open() < 2
                            did = 1
                        if did:
                            emitted = 1
                            budget -= 1

            # ---- fused projection + attention stream ----
            PIECE = 2 * TB  # 1024
            for g in range(NTB):
                if g % 2 == 0:
                    # one [128, 1024] piece per c-chunk covers t-blocks g, g+1
                    p0 = g * TB
                    for c in range(8):
                        x16 = x16p.tile([128, PIECE], F16, name="x16", tag="x16")
                        nc.gpsimd.dma_start(
                            out=x16,
                            in_=xt[c * 128:(c + 1) * 128, p0:p0 + PIECE],
                        )
                        x16s[c] = x16
                sl = slice((g % 2) * TB, (g % 2 + 1) * TB)
                psum_vk = pj_pool.tile([128, TB], F32, name="psum_vk", tag="pj")
                for c in range(8):
                    nc.tensor.matmul(
                        psum_vk,
                        wkv_sb[:, c * 128:(c + 1) * 128],
                        x16s[c][:, sl],
                        start=(c == 0),
                        stop=(c == 7),
                    )
                psum_q = pj_pool.tile([64, TB], F32, name="psum_q", tag="pj")
                for c in range(8):
                    nc.tensor.matmul(
                        psum_q,
                        wq_sb[:, c * H:(c + 1) * H],
                        x16s[c][:, sl],
                        start=(c == 0),
                        stop=(c == 7),
                    )
                nc.vector.tensor_copy(QT[:, g * TB:(g + 1) * TB], psum_q)
                nc.vector.tensor_copy(KT[:, g * TB:(g + 1) * TB], psum_vk[0:64, :])
                vt = vtp.tile([64, TB], F16)
                nc.vector.tensor_copy(vt, psum_vk[64:128, :])
                for j in range(4):
                    k = 4 * g + j
                    # V transpose on the DMA X-bar: [64,128] -> [128,64]
                    nc.sync.dma_start_transpose(
                        V16[:, k, 0:H], vt[:, j * 128:(j + 1) * 128]
                    )
                    nc.vector.tensor_copy(V8[:, k // 2, k % 2, 0:H],
                                          V16[:, k, 0:H])
                # attention filler: a few ready pairs per proj block
                emit_ready(g, 7 if g < NTB - 1 else -1)
            # drain any leftovers (g >= NTB unlocks the final PVs)
            emit_ready(NTB, -1)

    nc.compile()
    return nc


def get_nc():
    global _nc
    if _nc is None:
        _nc = _build()
    return _nc


def make_inputs(x, Wq, Wk, Wv):
    """Build the 8 per-core input maps."""
    x = np.asarray(x, dtype=np.float32)

    def pack_w(wt):
        # [C, M] (= W.T) -> [128, 8*M]: partition p, free c*M+m = wt[c*128+p, m]
        M = wt.shape[1]
        return np.ascontiguousarray(
            wt.reshape(8, 128, M).transpose(1, 0, 2).reshape(128, 8 * M)
        )

    wq_in = pack_w(np.asarray(Wq, np.float32).T)
    wkv_in = pack_w(
        np.concatenate(
            [np.asarray(Wk, np.float32).T, np.asarray(Wv, np.float32).T], axis=1
        )
    )
    p = np.arange(128, dtype=np.int64)[:, None]
    f = np.arange(TB, dtype=np.int64)[None, :]
    masks_by_s = []
    for s in range(2):
        m = np.concatenate(
            [((512 * s + f - 128 * j - p) >= 0).astype(np.float16) for j in range(8)],
            axis=1,
        )
        masks_by_s.append(np.ascontiguousarray(m))
    in_maps = []
    for core in range(NCORES):
        b, s = core // 2, core % 2
        in_maps.append(
            {
                "xt": np.ascontiguousarray(x[b].T),
                "wq": wq_in,
                "wkv": wkv_in,
                "masks": masks_by_s[s],
            }
        )
    return in_maps


def gather_output(results):
    """results: list of per-core {"out": [2048, 64]} -> full [B, T, H]."""
    O = np.empty((B, T, H), np.float32)
    for core in range(NCORES):
        b, s = core // 2, core % 2
        o = results[core]["out"]
        for i in range(NQB):
            g = 2 * i + s
            O[b, g * TB:(g + 1) * TB] = o[i * TB:(i + 1) * TB]
    return O


def kernel(x, Wq, Wk, Wv):
    nc = get_nc()
    in_maps = make_inputs(x, Wq, Wk, Wv)
    res = run_bass_kernel_spmd(nc, in_maps, list(range(NCORES)))
    return gather_output(res.results)


# revision 18
# speedup vs baseline: 1.3330x; 1.3330x over previous
"""Causal single-head attention (B=4, T=4096, C=1024, H=64) on 8 TRN2 NeuronCores.

Sharding: 2 cores per batch element; within a batch, the 8 query blocks of 512
rows are split by parity (core s owns blocks {s, s+2, s+4, s+6}).

v2.2:
  - x / weights cast f32->f16 in-flight by SWDGE DMA (no engine cast cost).
  - Attention is software-pipelined per kv PAIR: both S^T matmuls of a pair
    write one 2-bank PSUM tile, ONE exp covers the pair (fewer ACT
    instructions and semaphore round-trips on the critical recycle loop),
    then one fp8e4 DoubleRow P@V (far pairs) or two f16 P@Vs (near pairs,
    one paired causal-mask multiply split DVE/GpSimd).
  - V transposes ride the DMA X-bar (SBUF->SBUF transpose DMA) instead of
    PE transposes + DVE copies; the freed PSUM bank deepens the S pipeline.
  - PV emission lags S by 2 pairs round-robin across blocks so the PE always
    has ready work and the HAM clock stays warm; at most 2 blocks hold open
    PSUM accumulators (2 po banks).
"""

import numpy as np

import concourse.bacc as bacc
import concourse.bass as bass
import concourse.mybir as mybir
import concourse.tile as tile
from concourse.bass_utils import run_bass_kernel_spmd
from concourse.masks import make_identity

B, T, C, H = 4, 4096, 1024, 64
NCORES = 8
TB = 512                 # q/t block width
NTB = T // TB            # 8 t-blocks for projections
NQB = 4                  # local q blocks per core
NKVT = T // 128          # 32 kv tiles of 128
F32 = mybir.dt.float32
F16 = mybir.dt.float16
F8 = mybir.dt.float8e4

_nc = None


def _build():
    nc = bacc.Bacc("TRN2", target_bir_lowering=False, debug=False, num_devices=NCORES)
    xt = nc.dram_tensor("xt", [C, T], F32, kind="ExternalInput").ap()
    wq = nc.dram_tensor("wq", [128, 8 * H], F32, kind="ExternalInput").ap()
    wkv = nc.dram_tensor("wkv", [128, 8 * 2 * H], F32, kind="ExternalInput").ap()
    masks = nc.dram_tensor("masks", [128, 8 * TB], F16, kind="ExternalInput").ap()
    out = nc.dram_tensor("out", [NQB * TB, H], F32, kind="ExternalOutput").ap()

    with tile.TileContext(nc) as tc:
        pid = nc.partition_id(engines=[mybir.EngineType.PE])
        s = pid % 2
        with tc.tile_pool(name="persist", bufs=1) as persist, \
             tc.tile_pool(name="x16p", bufs=16) as x16p, \
             tc.tile_pool(name="vtp", bufs=2) as vtp, \
             tc.tile_pool(name="otp", bufs=2) as otp, \
             tc.tile_pool(name="obp", bufs=3) as obp, \
             tc.tile_pool(name="rcp", bufs=2) as rcp, \
             tc.tile_pool(name="ptp", bufs=6) as ptp, \
             tc.tile_pool(name="pt8p", bufs=6) as pt8p, \
             tc.tile_pool(name="pjp", bufs=2, space="PSUM") as pj_pool, \
             tc.tile_pool(name="psp", bufs=3, space="PSUM") as ps_pool, \
             tc.tile_pool(name="pvp", bufs=1, space="PSUM") as pv_pool, \
             tc.tile_pool(name="pop", bufs=2, space="PSUM") as po_pool:
            ident16 = persist.tile([128, 128], F16)
            make_identity(nc, ident16)
            wq_sb = persist.tile([128, 8 * H], F16)
            wkv_sb = persist.tile([128, 8 * 2 * H], F16)
            nc.gpsimd.dma_start(out=wq_sb, in_=wq)
            nc.gpsimd.dma_start(out=wkv_sb, in_=wkv)
            masks_sb = persist.tile([128, 8 * TB], F16)
            nc.scalar.dma_start(out=masks_sb, in_=masks)

            QT = persist.tile([64, T], F16)           # Q^T on partitions 0:64
            KT = persist.tile([64, T], F16)           # K^T on partitions 0:64
            # 80-wide rows: X-bar transpose dest must be 32B-aligned (160B stride)
            V16 = persist.tile([128, NKVT, 80], F16)
            # DoubleRow weights need the Ko step to be a multiple of 16 bytes:
            # pad each kv tile's [V | ones] row to 80 fp8 bytes (65:80 zero).
            V8 = persist.tile([128, NKVT // 2, 2, 80], F8)
            nc.vector.memset(V8, 0.0)
            # col 64 of each kv tile = 1.0 (row-sum column)
            nc.scalar.activation(
                V16[:, :, H],
                ident16[:, 0:NKVT],
                mybir.ActivationFunctionType.Copy,
                scale=0.0,
                bias=1.0,
            )
            for b8 in range(2):
                nc.scalar.activation(
                    V8[:, :, b8, H],
                    ident16[:, 0:NKVT // 2],
                    mybir.ActivationFunctionType.Copy,
                    scale=0.0,
                    bias=1.0,
                )

            # pre-warm the PE clock while the first x DMAs are in flight
            for w in range(8):
                psum_warm = ps_pool.tile([128, TB], F32, name="psum_warm",
                                         tag="ps")
                nc.tensor.matmul(
                    psum_warm[:, 0:128], ident16, ident16,
                    start=True, stop=True,
                )

            qoffs = [s * TB + i * 2 * TB for i in range(NQB)]
            x16s = [None] * 8

            # ---- attention machinery: S-phase / PV-phase software pipeline ----
            st = {
                "next_s": [0] * NQB,    # next pair to emit S+exp for
                "next_pv": [0] * NQB,   # next pair to emit PV for
                "po": [None] * NQB,
                "pt": {},               # (i, kp) -> pt pair tile
            }

            def npairs(i):
                return (8 * i + 8) // 2

            def emit_s(i, kp):
                nkv = 8 * i + 8
                far = (2 * kp + 1) < (nkv - 8)
                if far:
                    pt8 = pt8p.tile([128, 2, TB], F8, name="pt8", tag="pt8")
                    st["pt"][(i, kp)] = pt8
                else:
                    st["pt"][(i, kp)] = []
                for h in range(2):
                    k = 2 * kp + h
                    psum_s = ps_pool.tile([128, TB], F32, name="psum_s",
                                          tag="ps")
                    nc.tensor.matmul(
                        psum_s,
                        KT[:, k * 128:(k + 1) * 128],
                        QT[:, bass.ds(qoffs[i], TB)],
                        start=True,
                        stop=True,
                    )
                    if far:
                        nc.scalar.activation(
                            pt8[:, h, :], psum_s,
                            mybir.ActivationFunctionType.Exp, scale=0.125
                        )
                    else:
                        pt = ptp.tile([128, TB], F16, name="pt", tag="pt")
                        nc.scalar.activation(
                            pt, psum_s, mybir.ActivationFunctionType.Exp,
                            scale=0.125
                        )
                        j = k - (nkv - 8)
                        if j >= 0:
                            eng = nc.gpsimd if j < 4 else nc.vector
                            eng.tensor_mul(
                                pt, pt, masks_sb[:, j * TB:(j + 1) * TB]
                            )
                        st["pt"][(i, kp)].append(pt)

            def emit_pv(i, kp):
                nkv = 8 * i + 8
                far = (2 * kp + 1) < (nkv - 8)
                if kp == 0:
                    st["po"][i] = po_pool.tile([80, TB], F32, name="psum_o",
                                               tag="po")
                psum_o = st["po"][i]
                pt = st["pt"].pop((i, kp))
                if far:
                    nc.tensor.matmul(
                        psum_o,
                        V8[:, kp, :, :],
                        pt,
                        start=(kp == 0),
                        stop=False,
                        perf_mode=mybir.MatmulPerfMode.DoubleRow,
                    )
                else:
                    for h in range(2):
                        k = 2 * kp + h
                        nc.tensor.matmul(
                            psum_o[0:H + 1, :],
                            V16[:, k, 0:H + 1],
                            pt[h],
                            start=(k == 0),
                            stop=(k == nkv - 1),
                        )
                if 2 * kp + 1 == nkv - 1:
                    # epilogue: normalize + store this q block
                    ot = otp.tile([H + 1, TB], F16)
                    nc.vector.tensor_copy(ot, psum_o[0:H + 1, :])
                    for j2 in range(4):
                        psum_t = po_pool.tile([128, H + 1], F32, name="psum_t",
                                              tag="po")
                        nc.tensor.matmul(
                            psum_t,
                            ot[:, j2 * 128:(j2 + 1) * 128],
                            ident16[0:H + 1, 0:H + 1],
                            start=True,
                            stop=True,
                        )
                        rec = rcp.tile([128, 1], F32)
                        nc.vector.reciprocal(rec, psum_t[:, H:H + 1])
                        ob = obp.tile([128, H], F32)
                        nc.vector.tensor_scalar_mul(ob, psum_t[:, 0:H], rec)
                        nc.sync.dma_start(
                            out=out[i * TB + j2 * 128:i * TB + (j2 + 1) * 128, :],
                            in_=ob,
                        )

            def avail_g(i, kp):
                # q block i needs QT global block 2i+s (<= 2i+1); kv pair kp
                # needs proj t-block (2kp+1)//4.
                base = max(2 * i + 1, (2 * kp + 1) // 4)
                if i == 2:
                    base = max(base, 6)
                return base

            def n_open():
                return sum(
                    1 for i in range(NQB)
                    if 0 < st["next_pv"][i] < npairs(i)
                )

            def emit_ready(g, budget):
                emitted = 1
                while budget != 0 and emitted:
                    emitted = 0
                    for i in range(NQB):
                        if budget == 0:
                            break
                        did = 0
                        opened = 0 < st["next_pv"][i] < npairs(i)
                        may_open = opened or n_open() < 2
                        if (st["next_s"][i] < npairs(i)
                                and avail_g(i, st["next_s"][i]) <= g
                                and (st["next_s"][i] - st["next_pv"][i] < 2
                                     or may_open)):
                            emit_s(i, st["next_s"][i])
                            st["next_s"][i] += 1
                            did = 1
                        while may_open and (
                            st["next_pv"][i] < st["next_s"][i] - 2
                            or (st["next_s"][i] == npairs(i)
                                and g >= NTB
                                and st["next_pv"][i] < st["next_s"][i])
                        ):
                            emit_pv(i, st["next_pv"][i])
                            st["next_pv"][i] += 1
                            opened = st["next_pv"][i] < npairs(i)
                            may_open = opened or n_open() < 2
                            did = 1
                        if did:
                            emitted = 1
                            budget -= 1

            # ---- fused projection + attention stream ----
            PIECE = 2 * TB  # 1024
            for g in range(NTB):
                if g % 2 == 0:
                    # one [128, 1024] piece per c-chunk covers t-blocks g, g+1
                    p0 = g * TB
                    for c in range(8):
                        x16 = x16p.tile([128, PIECE], F16, name="x16", tag="x16")
                        nc.gpsimd.dma_start(
                            out=x16,
                            in_=xt[c * 128:(c + 1) * 128, p0:p0 + PIECE],
                        )
                        x16s[c] = x16
                sl = slice((g % 2) * TB, (g % 2 + 1) * TB)
                psum_vk = pj_pool.tile([128, TB], F32, name="psum_vk", tag="pj")
                for c in range(8):
                    nc.tensor.matmul(
                        psum_vk,
                        wkv_sb[:, c * 128:(c + 1) * 128],
                        x16s[c][:, sl],
                        start=(c == 0),
                        stop=(c == 7),
                    )
                psum_q = pj_pool.tile([64, TB], F32, name="psum_q", tag="pj")
                for c in range(8):
                    nc.tensor.matmul(
                        psum_q,
                        wq_sb[:, c * H:(c + 1) * H],
                        x16s[c][:, sl],
                        start=(c == 0),
                        stop=(c == 7),
                    )
                nc.vector.tensor_copy(QT[:, g * TB:(g + 1) * TB], psum_q)
                nc.vector.tensor_copy(KT[:, g * TB:(g + 1) * TB], psum_vk[0:64, :])
                vt = vtp.tile([64, TB], F16)
                nc.vector.tensor_copy(vt, psum_vk[64:128, :])
                for j in range(4):
                    k = 4 * g + j
                    psum_v = pv_pool.tile([128, H], F32)
                    nc.tensor.matmul(
                        psum_v,
                        vt[:, j * 128:(j + 1) * 128],
                        ident16[0:64, 0:64],
                        start=True,
                        stop=True,
                    )
                    nc.vector.tensor_copy(V16[:, k, 0:H], psum_v)
                    nc.vector.tensor_copy(V8[:, k // 2, k % 2, 0:H], psum_v)
                # attention filler: a few ready pairs per proj block
                emit_ready(g, 7 if g < NTB - 1 else -1)
            # drain any leftovers (g >= NTB unlocks the final PVs)
            emit_ready(NTB, -1)

    nc.compile()
    return nc


def get_nc():
    global _nc
    if _nc is None:
        _nc = _build()
    return _nc


def make_inputs(x, Wq, Wk, Wv):
    """Build the 8 per-core input maps."""
    x = np.asarray(x, dtype=np.float32)

    def pack_w(wt):
        # [C, M] (= W.T) -> [128, 8*M]: partition p, free c*M+m = wt[c*128+p, m]
        M = wt.shape[1]
        return np.ascontiguousarray(
            wt.reshape(8, 128, M).transpose(1, 0, 2).reshape(128, 8 * M)
        )

    wq_in = pack_w(np.asarray(Wq, np.float32).T)
    wkv_in = pack_w(
        np.concatenate(
            [np.asarray(Wk, np.float32).T, np.asarray(Wv, np.float32).T], axis=1
        )
    )
    p = np.arange(128, dtype=np.int64)[:, None]
    f = np.arange(TB, dtype=np.int64)[None, :]
    masks_by_s = []
    for s in range(2):
        m = np.concatenate(
            [((512 * s + f - 128 * j - p) >= 0).astype(np.float16) for j in range(8)],
            axis=1,
        )
        masks_by_s.append(np.ascontiguousarray(m))
    in_maps = []
    for core in range(NCORES):
        b, s = core // 2, core % 2
        in_maps.append(
            {
                "xt": np.ascontiguousarray(x[b].T),
                "wq": wq_in,
                "wkv": wkv_in,
                "masks": masks_by_s[s],
            }
        )
    return in_maps


def gather_output(results):
    """results: list of per-core {"out": [2048, 64]} -> full [B, T, H]."""
    O = np.empty((B, T, H), np.float32)
    for core in range(NCORES):
        b, s = core // 2, core % 2
        o = results[core]["out"]
        for i in range(NQB):
            g = 2 * i + s
            O[b, g * TB:(g + 1) * TB] = o[i * TB:(i + 1) * TB]
    return O


def kernel(x, Wq, Wk, Wv):
    nc = get_nc()
    in_maps = make_inputs(x, Wq, Wk, Wv)
    res = run_bass_kernel_spmd(nc, in_maps, list(range(NCORES)))
    return gather_output(res.results)
